# revision 27
# baseline (speedup 1.0000x reference)
"""Trainium2 Bass kernel for nn_CausalGraphModule (gnn_message_passing).

Strategy: time-shard state_sequence across 8 cores (each core owns ~16
timesteps of ALL batches, plus a 1-frame halo), so the per-timestep
presence/change booleans complete locally with no cross-core OR. The tiny
pairwise MLP is replicated; the [32,2] causal-increment partial is
AllReduced; preds output is t-sharded and concatenated on host.

Per-core scan (the memory-bound hot loop) per s-plane [128, 16*128]:
  - ACT: cast f32 -> int32
  - DVE: ttr(1 << v, or-reduce)  -> per-(h,t) presence bitmask column
         ttr(curr == prev, max-reduce) -> any-equal column
         reduce(min) of eq-map        -> all-equal column (has1 = min==0)
Finalize: expand mask bits -> [128, 16*32] 0/1, PE ones-matmul reduces the
partition (h) dim exactly in f32, reshape to t-on-partition, threshold,
tiny matmul -> [32,2] partial, AllReduce, assemble + sigmoid.
"""

import os
import sys

import numpy as np

if "/opt/trn_rl_repo" not in sys.path:
    sys.path.insert(0, "/opt/trn_rl_repo")

N = 32          # objects
D = 128         # d_model
B = 16          # batch
S = 128         # sequence
H = 128
W = 128
T = S - 1       # 127
NCORES = 8
TL = 16         # timesteps per core (core 7 has 15 valid)
SL = TL + 1     # frames per core incl. halo
PW = B * W      # plane free width = 2048
NP = N * N      # 1024 ordered pairs
NOD = N * (N - 1)   # 992 off-diagonal pairs
PREDW = NOD * 3     # 2976

_CACHE = {}


def _build_module():
    import concourse.bass as bass
    import concourse.bacc as bacc
    import concourse.tile as tile
    from concourse import mybir
    from concourse.alu_op_type import AluOpType as op
    from contextlib import ExitStack

    f32 = mybir.dt.float32
    i32 = mybir.dt.int32
    bf16 = mybir.dt.bfloat16
    AF = mybir.ActivationFunctionType
    X = mybir.AxisListType.X

    nc = bacc.Bacc()

    state = nc.declare_dram_parameter("state", [B, SL, H, W], f32, False)
    embp = nc.declare_dram_parameter("emb", [N, D], f32, False)
    w1p = nc.declare_dram_parameter("w1", [2 * D, D], f32, False)
    b1p = nc.declare_dram_parameter("b1", [D], f32, False)
    w2p = nc.declare_dram_parameter("w2", [D, 3], f32, False)
    b2p = nc.declare_dram_parameter("b2", [3], f32, False)
    causalp = nc.declare_dram_parameter("causal", [N, N], f32, False)
    tvalidp = nc.declare_dram_parameter("tvalid", [TL], f32, False)
    out_cm = nc.declare_dram_parameter("out_cm", [N, N], f32, True)
    out_preds = nc.declare_dram_parameter("out_preds", [TL, PREDW], f32, True)

    with ExitStack() as ctx:
        tc = ctx.enter_context(tile.TileContext(nc))
        const = ctx.enter_context(tc.tile_pool(name="const", bufs=1))
        planes = ctx.enter_context(tc.tile_pool(name="planes", bufs=6))
        vints = ctx.enter_context(tc.tile_pool(name="vints", bufs=3))
        scratch = ctx.enter_context(tc.tile_pool(name="scratch", bufs=3))
        acc = ctx.enter_context(tc.tile_pool(name="acc", bufs=1))
        small = ctx.enter_context(tc.tile_pool(name="small", bufs=1))
        psum = ctx.enter_context(tc.tile_pool(name="psum", bufs=1, space="PSUM"))
        dram = ctx.enter_context(tc.tile_pool(name="dram", bufs=1, space="DRAM"))

        # ---------------- constants ----------------
        ones32 = const.tile([128, PW], i32, tag="ones32")
        nc.vector.memset(ones32[:], 1)
        zcol32 = const.tile([128, 1], i32, tag="zcol32")
        nc.vector.memset(zcol32[:], 0)
        onescol = const.tile([128, 1], f32, tag="onescol")
        nc.vector.memset(onescol[:], 1.0)
        onescol16 = const.tile([128, 1], bf16, tag="onescol16")
        nc.vector.memset(onescol16[:], 1.0)

        # ---------------- MLP + preds (independent of scan) ----------------
        emb_sb = small.tile([N, D], f32, tag="emb")
        nc.sync.dma_start(out=emb_sb[:], in_=embp[:])
        w1a = small.tile([D, D], f32, tag="w1a")
        nc.sync.dma_start(out=w1a[:], in_=w1p[0:D])
        w1b = small.tile([D, D], f32, tag="w1b")
        nc.sync.dma_start(out=w1b[:], in_=w1p[D:2 * D])
        w2_sb = small.tile([D, 3], f32, tag="w2")
        nc.sync.dma_start(out=w2_sb[:], in_=w2p[:])
        b1_sb = small.tile([D, 1], f32, tag="b1")
        nc.sync.dma_start(out=b1_sb[:], in_=b1p[:].rearrange("(d o) -> d o", o=1))
        b2_sb = small.tile([3, 1], f32, tag="b2")
        nc.sync.dma_start(out=b2_sb[:], in_=b2p[:].rearrange("(d o) -> d o", o=1))

        # identity [N, N] for PE transpose
        idq = small.tile([N, N], i32, tag="idq")
        nc.gpsimd.iota(idq[:], pattern=[[1, N]], base=0, channel_multiplier=-1)
        identN = small.tile([N, N], f32, tag="identN")
        nc.vector.tensor_scalar(out=identN[:], in0=idq[:], scalar1=0, scalar2=None,
                                op0=op.is_equal)

        embT_ps = psum.tile([D, N], f32, tag="mlp_ps")
        nc.tensor.transpose(embT_ps[:], emb_sb[:], identN[:])
        embT = small.tile([D, N], f32, tag="embT")
        nc.vector.tensor_copy(embT[:], embT_ps[:])

        at_ps = psum.tile([D, N], f32, tag="mlp_ps")
        nc.tensor.matmul(at_ps[:], lhsT=w1a[:], rhs=embT[:], start=True, stop=True)
        at = small.tile([D, N], f32, tag="at")
        nc.vector.tensor_copy(at[:], at_ps[:])
        bt_ps = psum.tile([D, N], f32, tag="mlp_ps")
        nc.tensor.matmul(bt_ps[:], lhsT=w1b[:], rhs=embT[:], start=True, stop=True)
        bt = small.tile([D, N], f32, tag="bt")
        nc.vector.tensor_copy(bt[:], bt_ps[:])

        # out1T[d, p] = AT[d, p//32] + BT[d, p%32]  (AP broadcast tricks)
        out1T = small.tile([D, NP], f32, tag="out1T")
        at_rep = at[:].rearrange("d (i o) -> d i o", o=1).broadcast_to((D, N, N))
        bt_rep = bt[:].rearrange("d (o j) -> d o j", o=1).broadcast_to((D, N, N))
        nc.vector.tensor_tensor(out=out1T[:], in0=at_rep, in1=bt_rep, op=op.add)

        hT = small.tile([D, NP], f32, tag="hT")
        nc.scalar.activation(hT[:], out1T[:], AF.Relu, bias=b1_sb[:, 0:1])

        mechT = small.tile([3, NP], f32, tag="mechT")
        for half in range(2):
            mps = psum.tile([3, NP // 2], f32, tag="scratch_ps")
            nc.tensor.matmul(mps[:], lhsT=w2_sb[:],
                             rhs=hT[:, half * (NP // 2):(half + 1) * (NP // 2)],
                             start=True, stop=True)
            nc.scalar.activation(mechT[:, half * (NP // 2):(half + 1) * (NP // 2)],
                                 mps[:], AF.Identity, bias=b2_sb[:, 0:1])

        # compact off-diagonal pairs (drop p = 33*i), one partition-0 row per
        # channel so PE can broadcast them
        ones_row = const.tile([1, TL], f32, tag="ones_row")
        nc.vector.memset(ones_row[:], 1.0)
        rep = small.tile([TL, PREDW], f32, tag="rep")
        HB = NOD // 2  # 496
        for c in range(3):
            chan = small.tile([1, NOD], f32, tag=f"chan{c}")
            src = mechT[c:c + 1, 1:NP].rearrange("p (k j) -> p k j",
                                                 j=N + 1)[:, :, 0:N]
            nc.sync.dma_start(out=chan[:], in_=src.opt())
            # broadcast to TL partitions via PE, interleave (c fastest) into
            # rep with strided DVE writes
            for h in range(2):
                bc_ps = psum.tile([TL, HB], f32, tag="bc_ps")
                nc.tensor.matmul(bc_ps[:], lhsT=ones_row[:],
                                 rhs=chan[0:1, h * HB:(h + 1) * HB],
                                 start=True, stop=True)
                lo = 3 * h * HB + c
                nc.vector.tensor_copy(rep[:, lo:lo + 3 * HB - 2:3], bc_ps[:])
        nc.sync.dma_start(out=out_preds[:], in_=rep[:])

        # ---------------- the scan ----------------
        colmask = acc.tile([128, TL], i32, tag="colmask")
        # eq-count accumulator: per t a [1, 128] PSUM region, 16 accumulating
        # b-block matmuls sum the whole eq-map exactly (integer counts < 2^24)
        eqcnt = psum.tile([1, TL * 128], f32, tag="eqcnt")

        BG = 8  # batches per DMA instruction
        prev_pl = None
        for s in range(SL):
            pl = planes.tile([128, PW], f32, tag="plane")
            for b0 in range(0, B, BG):
                src = state[b0:b0 + BG, s].rearrange("b h w -> h b w")
                dst = pl[:, b0 * W:(b0 + BG) * W].rearrange(
                    "p (b w) -> p b w", b=BG)
                nc.sync.dma_start(out=dst, in_=src)
            b16 = vints.tile([128, PW], bf16, tag="b16")
            nc.scalar.copy(b16[:], pl[:])
            if s < TL:
                vi = vints.tile([128, PW], i32, tag="vint")
                nc.scalar.copy(vi[:], pl[:])
                scr = scratch.tile([128, PW], i32, tag="scr32")
                nc.vector.tensor_tensor(out=scr[:], in0=ones32[:], in1=vi[:],
                                        op=op.logical_shift_left)
                nc.vector.tensor_reduce(out=colmask[:, s:s + 1], in_=scr[:],
                                        axis=X, op=op.bitwise_or)
            if s >= 1:
                t = s - 1
                scrf = scratch.tile([128, PW], bf16, tag="scrf")
                nc.vector.tensor_tensor(out=scrf[:], in0=b16[:],
                                        in1=prev_b16[:], op=op.is_equal)
                for b in range(B):
                    nc.tensor.matmul(eqcnt[0:1, t * 128:(t + 1) * 128],
                                     lhsT=onescol16[:],
                                     rhs=scrf[:, b * W:(b + 1) * W],
                                     start=(b == 0), stop=(b == B - 1),
                                     skip_group_check=True)
            prev_pl = pl
            prev_b16 = b16

        # ---------------- finalize ----------------
        # expand colmask bits -> [128, TL*32] 0/1 f32 (t-major, i-minor)
        iotai = small.tile([128, TL * N], i32, tag="iotai")
        nc.gpsimd.iota(iotai[:], pattern=[[0, TL], [1, N]], base=0,
                       channel_multiplier=0)
        shifted = small.tile([128, TL * N], i32, tag="shifted")
        cm_rep = colmask[:].rearrange("p (t o) -> p t o", o=1).broadcast_to((128, TL, N))
        nc.vector.tensor_tensor(out=shifted[:], in0=cm_rep, in1=iotai[:],
                                op=op.logical_shift_right)
        bits = small.tile([128, TL * N], i32, tag="bits")
        nc.vector.tensor_scalar(out=bits[:], in0=shifted[:], scalar1=1,
                                scalar2=None, op0=op.bitwise_and)
        stk = small.tile([128, TL * N], f32, tag="stk")
        nc.vector.tensor_copy(stk[:], bits[:])

        # PE reduces partition dim exactly (all sums < 2^24, exact in f32)
        red_sb = small.tile([1, TL * N], f32, tag="red_sb")
        ps = psum.tile([1, TL * N], f32, tag="scratch_ps")
        nc.tensor.matmul(ps[:], lhsT=onescol[:], rhs=stk[:], start=True,
                         stop=True)
        nc.vector.tensor_copy(red_sb[:], ps[:])

        # total eq-count per t from the PSUM accumulator
        cnt_row = small.tile([1, TL], f32, tag="cnt_row")
        nc.vector.tensor_reduce(
            out=cnt_row[:],
            in_=eqcnt[0:1, :].rearrange("p (t w) -> p t w", t=TL),
            axis=X, op=op.add)

        # reshape row -> t-on-partition tiles
        pres_t = small.tile([TL, N], f32, tag="pres_t")
        nc.sync.dma_start(
            out=pres_t[:],
            in_=red_sb[0:1, :].rearrange("p (t i) -> p t i", t=TL))
        cnt_t = small.tile([TL, 1], f32, tag="cnt_t")
        nc.sync.dma_start(
            out=cnt_t[:],
            in_=cnt_row[0:1, :].rearrange("p (t o) -> p t o", o=1))

        presf = small.tile([TL, N], f32, tag="presf")
        nc.vector.tensor_scalar(out=presf[:], in0=pres_t[:], scalar1=0.5,
                                scalar2=None, op0=op.is_gt)
        tv = small.tile([TL, 1], f32, tag="tv")
        nc.sync.dma_start(out=tv[:], in_=tvalidp[:].rearrange("(t o) -> t o", o=1))
        # has0 = any cell equal (count > 0); has1 = any cell diff (count < all)
        NCELLS = float(128 * PW)
        chgf = small.tile([TL, 2], f32, tag="chgf")
        nc.vector.tensor_scalar(out=chgf[:, 0:1], in0=cnt_t[:], scalar1=0.5,
                                scalar2=tv[:, 0:1], op0=op.is_gt, op1=op.mult)
        nc.vector.tensor_scalar(out=chgf[:, 1:2], in0=cnt_t[:],
                                scalar1=NCELLS - 0.5, scalar2=tv[:, 0:1],
                                op0=op.is_lt, op1=op.mult)

        inc_ps = psum.tile([N, 2], f32, tag="scratch_ps")
        nc.tensor.matmul(inc_ps[:], lhsT=presf[:], rhs=chgf[:], start=True,
                         stop=True)
        inc_sb = small.tile([N, 2], f32, tag="inc_sb")
        nc.vector.tensor_copy(inc_sb[:], inc_ps[:])

        cc_in = dram.tile([N, 2], f32, tag="cc_in")
        cc_out = dram.tile([N, 2], f32, tag="cc_out")
        nc.sync.dma_start(out=cc_in[:], in_=inc_sb[:])
        nc.gpsimd.collective_compute(
            "AllReduce", op.add, replica_groups=[list(range(NCORES))],
            ins=[cc_in.opt()], outs=[cc_out.opt()])
        incsum = small.tile([N, 2], f32, tag="incsum")
        nc.sync.dma_start(out=incsum[:], in_=cc_out[:])

        # cm = sigmoid(causal + 0.01 * incsum * offdiag-mask on cols 0,1)
        causal_sb = small.tile([N, N], f32, tag="causal_sb")
        nc.sync.dma_start(out=causal_sb[:], in_=causalp[:])
        dm = small.tile([N, 2], i32, tag="dm")
        nc.gpsimd.iota(dm[:], pattern=[[-1, 2]], base=0, channel_multiplier=1)
        dmf = small.tile([N, 2], f32, tag="dmf")
        nc.vector.tensor_scalar(out=dmf[:], in0=dm[:], scalar1=0, scalar2=0.01,
                                op0=op.not_equal, op1=op.mult)
        adj = small.tile([N, 2], f32, tag="adj")
        nc.vector.tensor_tensor(out=adj[:], in0=incsum[:], in1=dmf[:], op=op.mult)
        nc.vector.tensor_tensor(out=causal_sb[:, 0:2], in0=causal_sb[:, 0:2],
                                in1=adj[:], op=op.add)
        cmout = small.tile([N, N], f32, tag="cmout")
        nc.scalar.activation(cmout[:], causal_sb[:], AF.Sigmoid)
        nc.sync.dma_start(out=out_cm[:], in_=cmout[:])

    nc.compile()
    return nc


def _get_module():
    if "nc" not in _CACHE:
        _CACHE["nc"] = _build_module()
    return _CACHE["nc"]


def _shard_inputs(state_sequence, emb, W1, b1, W2, b2, causal_matrix):
    state_sequence = np.ascontiguousarray(state_sequence, dtype=np.float32)
    common = {
        "emb": np.ascontiguousarray(emb, dtype=np.float32),
        "w1": np.ascontiguousarray(W1, dtype=np.float32),
        "b1": np.ascontiguousarray(b1, dtype=np.float32),
        "w2": np.ascontiguousarray(W2, dtype=np.float32),
        "b2": np.ascontiguousarray(b2, dtype=np.float32),
        "causal": np.ascontiguousarray(causal_matrix, dtype=np.float32),
    }
    in_maps = []
    for k in range(NCORES):
        s0 = TL * k
        if s0 + SL <= S:
            sh = state_sequence[:, s0:s0 + SL]
        else:  # last core: pad with a copy of the final frame
            sh = np.concatenate(
                [state_sequence[:, s0:S], state_sequence[:, S - 1:S]], axis=1)
        tval = np.ones(TL, np.float32)
        nvalid = min(TL, T - s0)
        tval[nvalid:] = 0.0
        in_maps.append(dict(common, state=np.ascontiguousarray(sh),
                            tvalid=tval))
    return in_maps


def kernel(state_sequence, emb, W1, b1, W2, b2, causal_matrix):
    from concourse.bass_utils import run_bass_kernel_spmd

    nc = _get_module()
    in_maps = _shard_inputs(state_sequence, emb, W1, b1, W2, b2, causal_matrix)
    trace = bool(int(os.environ.get("KERNEL_TRACE", "0")))
    res = run_bass_kernel_spmd(nc, in_maps, core_ids=list(range(NCORES)),
                               trace=trace)
    if trace and res.exec_time_ns is not None:
        print(f"HW exec time: {res.exec_time_ns} ns")
        _CACHE["exec_time_ns"] = res.exec_time_ns

    cm = res.results[0]["out_cm"]
    rows = []
    for k in range(NCORES):
        nvalid = min(TL, T - TL * k)
        rows.append(res.results[k]["out_preds"][:nvalid])
    preds = np.concatenate(rows, axis=0).reshape(T * NOD, 1, 3)
    return np.asarray(cm, np.float32), np.ascontiguousarray(preds, np.float32)
